# revision 14
# baseline (speedup 1.0000x reference)
"""Decoder-only attention block (QKV proj + MHA + out proj) on 8 TRN2 cores.

Sharding: core c -> (batch b = c//4, head-group g = c%4). Tensor-parallel over
heads (4 of 16 heads per core), data-parallel over batch (2). Each core
computes a partial c_proj over its 512 input features; host reduces the 4
partials per batch and adds biases.

Self-contained: hardcodes B=2, S=2048, D=2048, H=16.
"""

import os

import numpy as np

NPF16 = np.float16

import concourse.bass as bass
import concourse.bacc as bacc
import concourse.tile as tile
from concourse import mybir
import concourse.bass_utils as bass_utils
import concourse.bass_isa as bass_isa
from concourse.bass_interp import get_hw_module

B, S, D = 2, 2048, 2048
H, DH = 16, 128
N_CORES = 8
HL = H // 4            # 4 heads per core
FL = HL * DH           # 512 local features per core
KT = D // 128          # 16 contraction tiles
TT = S // 128          # 16 token tiles
QB = S // 512          # 4 token blocks
SCALE = 1.0 / float(np.sqrt(DH))

F16 = mybir.dt.float16
F32 = mybir.dt.float32

# Stash of the last BassKernelResults (for the local test harness only).
LAST_RESULTS = None
_PROG_CACHE = {}


def _build_program(use_mask):

    nc = bacc.Bacc("TRN2", target_bir_lowering=False, debug=False,
                   num_devices=N_CORES)

    # Host-repacked layouts so each logical input group is ONE contiguous
    # DMA: wqk is f-major ([p, f, kt, j]), xt is token-block-major
    # ([p, tb, kt, s']), wv is kt-major ([p, kt, j]).
    xt_d = nc.dram_tensor("xt", [128, QB * KT * 512], F16,
                          kind="ExternalInput")
    wqk_d = nc.dram_tensor("wqk", [128, 8 * KT * 128], F16,
                           kind="ExternalInput")
    wv_d = nc.dram_tensor("wv", [128, KT * 512], F16, kind="ExternalInput")
    wp_d = nc.dram_tensor("wp", [FL, D], F16, kind="ExternalInput")
    bqk_d = nc.dram_tensor("bqk", [128, 8], F32, kind="ExternalInput")
    kb_d = nc.dram_tensor("kb", [128, KT], F32, kind="ExternalInput")
    ones_d = nc.dram_tensor("ones", [128, 128], F16, kind="ExternalInput")
    out_d = nc.dram_tensor("out", [S, D], F16, kind="ExternalOutput")

    xt_ap, wqk_ap, wv_ap, wp_ap = xt_d.ap(), wqk_d.ap(), wv_d.ap(), wp_d.ap()
    bqk_ap, kb_ap, ones_ap, out_ap = (bqk_d.ap(), kb_d.ap(), ones_d.ap(),
                                      out_d.ap())

    with tile.TileContext(nc) as tc, tc.tile_pool(name="pers", bufs=1) as pers:
        # ---- persistent tiles (live across phases) ----
        qt = [pers.tile([128, S], F16, tag=f"qt{h}", name=f"qt{h}")
              for h in range(HL)]
        ktt = [pers.tile([128, S], F16, tag=f"kt{h}", name=f"ktt{h}")
               for h in range(HL)]
        ot = [pers.tile([128, S], F16, tag=f"ot{h}", name=f"ot{h}")
              for h in range(HL)]
        # V rows: per 128-token tile, 4 heads packed side by side [128, 512]
        vrow = [pers.tile([128, FL], F16, tag=f"v{t}", name=f"v{t}")
                for t in range(TT)]
        wp_sb = [pers.tile([128, D], F16, tag=f"wp{h}", name=f"wp{h}")
                 for h in range(HL)]
        bqk_sb = pers.tile([128, 8], F32, tag="bqk", name="bqk_sb")
        kb_sb = pers.tile([128, KT], F32, tag="kb", name="kb_sb")
        ones_sb = pers.tile([128, 128], F16, tag="ones", name="ones_sb")

        # (small const DMAs are issued after the first big transfers below;
        # bqk is first needed ~18us in, ones not until phase 2)

        # ---- phase 1: QKV projection (token-block outer) ----
        with (
            tc.tile_pool(name="p1in", bufs=1) as p1in,
            tc.tile_pool(name="p1ps", bufs=1, space="PSUM") as p1ps,
        ):
            wqkf = [p1in.tile([128, KT * 128], F16, tag=f"wqkf{f}",
                              name=f"wqkf{f}") for f in range(8)]
            # token block 0 is split into two single-writer tiles (kt 0-7 /
            # kt 8-15) so the first QK chain starts on the first half while
            # the second half is still in flight
            xtb0 = [p1in.tile([128, KT * 256], F16, tag=f"xtb0{i}",
                              name=f"xtb0{i}") for i in range(2)]
            xtb = [None] + [p1in.tile([128, KT * 512], F16, tag=f"xtb{tb}",
                                      name=f"xtb{tb}")
                            for tb in range(1, QB)]
            wv_sb = p1in.tile([128, KT * 512], F16, tag="wv", name="wv_sb")

            def x_sl(tb, kt, lo, hi):
                if tb == 0:
                    t_ = xtb0[kt // 8]
                    base = (kt % 8) * 512
                    return t_[:, base + lo:base + hi]
                return xtb[tb][:, kt * 512 + lo:kt * 512 + hi]

            # DMA order: f=0 weights -> first token block (split in 4 so the
            # first QK chain starts on its leading kt chunks while the rest
            # streams) -> consts -> remaining weights -> wv -> remaining
            # token blocks -> wp. The first QK chain needs only ~2.5 MB;
            # each later group lands well before the compute that uses it.
            nc.sync.dma_start(wqkf[0][:], wqk_ap[:, 0:KT * 128])
            nc.sync.dma_start(xtb0[0][:], xt_ap[:, 0:KT * 256])
            nc.sync.dma_start(xtb0[1][:], xt_ap[:, KT * 256:KT * 512])
            nc.sync.dma_start(bqk_sb[:], bqk_ap[:])
            nc.sync.dma_start(kb_sb[:], kb_ap[:])
            nc.sync.dma_start(ones_sb[:], ones_ap[:])
            for f in range(1, 8):
                nc.sync.dma_start(wqkf[f][:],
                                  wqk_ap[:, f * KT * 128:(f + 1) * KT * 128])
            nc.sync.dma_start(wv_sb[:], wv_ap[:])
            for tb in range(1, QB):
                nc.sync.dma_start(xtb[tb][:],
                                  xt_ap[:, tb * KT * 512:(tb + 1) * KT * 512])
            for h in range(HL):
                nc.sync.dma_start(wp_sb[h][:], wp_ap[h * 128:(h + 1) * 128, :])

            for tb in range(QB):
                # Q^T and K^T columns for this token block
                for f in range(8):
                    ps = p1ps.tile([128, 512], F32, tag="psqk", bufs=4,
                                   name="psqk")
                    for kt in range(KT):
                        nc.tensor.matmul(
                            ps[:],
                            wqkf[f][:, kt * 128:(kt + 1) * 128],
                            x_sl(tb, kt, 0, 512),
                            start=(kt == 0), stop=(kt == KT - 1),
                        )
                    dest = qt[f] if f < HL else ktt[f - HL]
                    nc.scalar.add(dest[:, tb * 512:(tb + 1) * 512], ps[:],
                                  bqk_sb[:, f:f + 1])
                # V rows for this token block: [tok, feat], x^T stationary
                for tt_ in range(4):
                    t = tb * 4 + tt_
                    psv = p1ps.tile([128, FL], F32, tag="psv", bufs=2,
                                    name="psv")
                    for kt in range(KT):
                        nc.tensor.matmul(
                            psv[:],
                            x_sl(tb, kt, tt_ * 128, (tt_ + 1) * 128),
                            wv_sb[:, kt * 512:(kt + 1) * 512],
                            start=(kt == 0), stop=(kt == KT - 1),
                        )
                    nc.vector.tensor_copy(vrow[t][:], psv[:])

        # ---- phase 2: attention, head-windows flattened over (qb, h) ----
        # Per window: scores^T pairs stream into a 2-tile pss ring, each
        # followed by one exp [128,1024] on ScalarE; AV matmuls (V^T @ E^T,
        # psot accumulation) trail the exp stream by two pairs so TensorE
        # always has ready work. Softmax denominator: serial f16 accumulator
        # on DVE (low post-exp latency) -> ones-matmul partition reduce
        # (PSUM broadcast, allocated through the pss ring) -> DVE
        # reciprocal; normalization fused into psot evacuation. The
        # denominator matmul + evacuation of window w are emitted inside
        # window w+1. Partial c_proj for token block qb-1 rides along, one
        # token tile per window.
        with (
            tc.tile_pool(name="p2", bufs=1) as p2,
            tc.tile_pool(name="p2ps", bufs=1, space="PSUM") as p2ps,
        ):
            pend = None  # (dn_tile, psot_tile, qb, h) awaiting dnr/rcp/evac

            def flush_pend():
                dn, psot_p, pqb, ph = pend
                dnr = p2ps.tile([128, 512], F32, tag="psp", bufs=2,
                                name="dnr")
                nc.tensor.matmul(dnr[:], ones_sb[:], dn[:],
                                 start=True, stop=True,
                                 skip_group_check=True)
                rcp = p2.tile([128, 512], F32, tag="rcp", bufs=2, name="rcp")
                nc.vector.reciprocal_approx_fast(rcp[:], dnr[:])
                nc.vector.tensor_mul(
                    ot[ph][:, pqb * 512:(pqb + 1) * 512], psot_p[:], rcp[:])

            def cproj_tile(t, st):
                for nb in range(4):
                    psp = p2ps.tile([128, 512], F32, tag="psp", bufs=2,
                                    name="psp")
                    for h in range(HL):
                        nc.tensor.matmul(
                            psp[:],
                            ot[h][:, t * 128:(t + 1) * 128],
                            wp_sb[h][:, nb * 512:(nb + 1) * 512],
                            start=(h == 0), stop=(h == HL - 1),
                            skip_group_check=True,
                        )
                    nc.vector.tensor_copy(st[:, nb * 512:(nb + 1) * 512],
                                          psp[:])
                    nc.sync.dma_start(
                        out_ap[t * 128:(t + 1) * 128,
                               nb * 512:(nb + 1) * 512],
                        st[:, nb * 512:(nb + 1) * 512])

            for hh in range(QB * HL):
                qb, h = divmod(hh, HL)
                e_tiles = [None] * (KT // 2)
                acc = p2.tile([128, 1024], F16, tag="acc", bufs=2, name="acc")
                psot = p2ps.tile([128, 512], F32, tag="pot", bufs=2,
                                 name="psot")

                def av_pair(p):
                    for half in range(2):
                        kt = 2 * p + half
                        nc.tensor.matmul(
                            psot[:],
                            vrow[kt][:, h * 128:(h + 1) * 128],
                            e_tiles[p][:, half * 512:(half + 1) * 512],
                            start=(kt == 0), stop=(kt == KT - 1),
                            skip_group_check=True,
                        )

                for p in range(KT // 2):
                    pss = p2ps.tile([128, 1024], F32, tag="pss", bufs=2,
                                    name="pss")
                    for half in range(2):
                        kt = 2 * p + half
                        nc.tensor.matmul(
                            pss[:, half * 512:(half + 1) * 512],
                            ktt[h][:, kt * 128:(kt + 1) * 128],
                            qt[h][:, qb * 512:(qb + 1) * 512],
                            start=True, stop=True,
                        )
                    e = p2.tile([128, 1024], F16, tag=f"e{p}", bufs=3,
                                name=f"e{p}")
                    nc.scalar.activation(
                        e[:], pss[:], mybir.ActivationFunctionType.Exp,
                        scale=SCALE,
                    )
                    if use_mask:
                        for half in range(2):
                            kt = 2 * p + half
                            sl = e[:, half * 512:(half + 1) * 512]
                            nc.vector.tensor_scalar_mul(
                                sl, sl, kb_sb[:, kt:kt + 1])
                    e_tiles[p] = e
                    # running denominator sum (f16, DVE 2x mode)
                    if p == 1:
                        nc.vector.tensor_add(acc[:], e_tiles[0][:],
                                             e_tiles[1][:])
                    elif p > 1:
                        nc.vector.tensor_add(acc[:], acc[:], e[:])
                    # window w-1 epilogue sits here so its denominator
                    # matmul lands mid-stream, never at an engine-queue head
                    if p == 1 and pend is not None:
                        flush_pend()
                    if p >= 2:
                        av_pair(p - 2)
                av_pair(KT // 2 - 2)
                av_pair(KT // 2 - 1)
                dn = p2.tile([128, 512], F16, tag="dn", bufs=2, name="dn")
                nc.vector.tensor_add(dn[:], acc[:, 0:512], acc[:, 512:1024])
                pend = (dn, psot, qb, h)

                # c_proj for token block qb-1, one token tile per window
                if qb > 0:
                    t = (qb - 1) * 4 + h
                    st = p2.tile([128, D], F16, tag="stage", bufs=2,
                                 name="stage")
                    cproj_tile(t, st)

            flush_pend()
            for t in range((QB - 1) * 4, QB * 4):
                st = p2.tile([128, D], F16, tag="stage", bufs=2, name="stage")
                cproj_tile(t, st)

    nc.compile()
    nc.m = get_hw_module(nc.m)
    return nc


def kernel(hidden_states, attention_mask, w_attn, b_attn, w_proj, b_proj):
    global LAST_RESULTS
    hidden_states = np.asarray(hidden_states, dtype=np.float32)
    attention_mask = np.asarray(attention_mask, dtype=np.float32)
    w_attn = np.asarray(w_attn, dtype=np.float32)
    b_attn = np.asarray(b_attn, dtype=np.float32)
    w_proj = np.asarray(w_proj, dtype=np.float32)
    b_proj = np.asarray(b_proj, dtype=np.float32)

    use_mask = bool((attention_mask != 1.0).any())
    key = ("prog", use_mask)
    if key not in _PROG_CACHE:
        _PROG_CACHE[key] = _build_program(use_mask)
    nc = _PROG_CACHE[key]

    ones128 = np.ones((128, 128), dtype=NPF16)
    in_maps = []
    for c in range(N_CORES):
        b, g = divmod(c, 4)
        xt = hidden_states[b].T.astype(NPF16)        # [D, S]
        # [p, tb, kt, s'] layout: one contiguous DMA per token block
        xt2 = np.ascontiguousarray(
            xt.reshape(KT, 128, QB, 512).transpose(1, 2, 0, 3)
            .reshape(128, QB * KT * 512))
        wq = w_attn[:, g * FL:(g + 1) * FL]
        wk = w_attn[:, D + g * FL:D + (g + 1) * FL]
        wv = w_attn[:, 2 * D + g * FL:2 * D + (g + 1) * FL]
        wqk = np.concatenate([wq, wk], axis=1).astype(NPF16)  # [D, 1024]
        # [p, f, kt, j] layout: one contiguous DMA per output feature column
        wqk2 = np.ascontiguousarray(
            wqk.reshape(KT, 128, 8, 128).transpose(1, 2, 0, 3)
            .reshape(128, 8 * KT * 128))
        wv2 = np.ascontiguousarray(
            wv.astype(NPF16).reshape(KT, 128, FL).transpose(1, 0, 2)
            .reshape(128, KT * FL))
        wp = np.ascontiguousarray(w_proj[g * FL:(g + 1) * FL, :]).astype(NPF16)
        bq = b_attn[g * FL:(g + 1) * FL]
        bk = b_attn[D + g * FL:D + (g + 1) * FL]
        bqk = np.ascontiguousarray(
            np.concatenate([bq, bk]).reshape(8, 128).T).astype(np.float32)
        kb = np.ascontiguousarray(
            attention_mask[b].reshape(KT, 128).T).astype(np.float32)
        in_maps.append({
            "xt": xt2,
            "wqk": wqk2,
            "wv": wv2,
            "wp": wp,
            "bqk": bqk,
            "kb": kb,
            "ones": ones128,
        })

    if not os.environ.get("KERNEL_ALLOW_TRACE"):
        os.environ["BASS_NEVER_TRACE"] = "1"
    try:
        res = bass_utils.run_bass_kernel_spmd(nc, in_maps,
                                              list(range(N_CORES)))
    except Exception:
        # Transient NRT failures can leave the axon device wedged; reset it
        # once and retry. If the reset path is unavailable, the retry's own
        # failure propagates.
        try:
            import ctypes

            import jax

            jax.devices()
            _lib = ctypes.CDLL("/opt/axon/libaxon_pjrt.so")
            _lib.axon_reset.restype = ctypes.c_int64
            _lib.axon_reset()
        except Exception:
            pass
        res = bass_utils.run_bass_kernel_spmd(nc, in_maps,
                                              list(range(N_CORES)))
    LAST_RESULTS = res

    # host reduce: sum the 4 head-group partials per batch, add biases.
    # V-bias contribution: rows of A sum to 1, so each core's O gains b_v
    # per row; through c_proj that's a constant row b_v @ w_proj_slice.
    out = np.zeros((B, S, D), dtype=np.float32)
    for c in range(N_CORES):
        b, g = divmod(c, 4)
        out[b] += res.results[c]["out"].astype(np.float32)
    bias_row = b_proj.astype(np.float64).copy()
    for g in range(4):
        bv = b_attn[2 * D + g * FL:2 * D + (g + 1) * FL].astype(np.float64)
        bias_row += bv @ w_proj[g * FL:(g + 1) * FL, :].astype(np.float64)
    out += bias_row.astype(np.float32)[None, None, :]
    return out
